# revision 3
# baseline (speedup 1.0000x reference)
"""NodeContrastiveLoss on 8 Trainium2 NeuronCores (Bass/Tile), v2.

loss = mean_i[ -(z1n_i . z2n_i)/tau
               + log( sum_j exp((z1n_i . z2n_j)/tau)
                    + sum_{j!=i} exp((z1n_i . z1n_j)/tau) ) ]

Sharding: z1 query rows split 8 ways (2048 rows/core); every core builds the
full normalized key matrices z1n^T, z2n^T in SBUF (bf16) and computes its row
block of the similarity logits with PE matmuls.

v2 change vs v1: the exp+row-sum of each [128 x 2048] PSUM chunk is SPLIT
between two engines working concurrently on disjoint column ranges:
  - ACT: native Exp activation with accum_out on cols [0:ACT_W]
  - DVE: Schraudolph-style bf16 bit-trick exp on cols [ACT_W:2048]:
      op1: int16 <- A_EXP*s + B_EXP   (1x, PSUM f32 -> SBUF int16)
      op2: bitcast int16 as bf16, re-copy with accum_out row-sum (4x mode)
    bitcast(int16(A*s+B)) == 2^(s*log2e/tau + frac-interp) ~ exp(s/tau) with
    |rel err| <= 4.2% per element and ~0 mean bias (B calibrated); errors
    wash out in the 32768-term row sums and the 16384-row mean.
Key prep (squares for row norms, normalize-scale to bf16) moves to GpSimd +
batched DVE reduces so the DVE's cycles go to the exp stream. A burst of
dummy matmuls at t=0 warms the PE HAM clock gate (cold PE = 1.2 GHz would
otherwise gate the pipeline).

The z1-z1 diagonal is removed by subtracting exp(||z1n_i||^2/tau) computed
from the same bf16 values the PE consumes - via ACT Exp for q-tiles whose
diagonal falls in the ACT columns (q<10) and via the identical DVE bit-trick
transform for q>=10 (ACT_W=1280 is a multiple of 128 so each q-tile's
diagonal lies entirely on one side).
"""

import os
import numpy as np

N, D = 16384, 128
TAU = 0.07
NCORES = 8
NQ = N // NCORES          # 2048 query rows per core
P = 128
QT = NQ // P              # 16 query tiles per core
GROUP = 32                # row tiles per staging group (4096 rows)
CHUNK = 2048              # keys per chunk (4 PSUM banks)
SUB = 512                 # matmul moving free dim (1 PSUM bank)
NGRP = N // (GROUP * P)   # 4 groups per key matrix
NCHUNKS = 2 * N // CHUNK  # 16 global chunks per q-tile (z2 then z1)

ACT_W = 1280              # ACT exp columns per chunk (multiple of 128)
DVE_W = CHUNK - ACT_W     # 768 DVE bit-trick columns per chunk
QSPLIT = ACT_W // P       # q-tiles 0..QSPLIT-1 have their diagonal in ACT cols

LOG2E = 1.4426950408889634
A_EXP = 128.0 * LOG2E / TAU          # 2638.0709...
B_EXP = 16248.888                    # 128*127 - 7.112 (zero-bias calibrated)

NWARM = 28                # dummy matmuls to warm the PE HAM clock gate

_CACHE = {}


def _split_excess_waits(nc, mybir):
    """walrus in this env supports 1 sync-wait per instruction (2 for
    EventSemaphore); move excess waits onto injected same-engine NoOps."""
    n = 0
    for f in nc.m.functions:
        for bb in f.blocks:
            new_insts = None
            for idx, inst in enumerate(bb.instructions):
                si = getattr(inst, "sync_info", None)
                waits = list(si.on_wait) if si is not None and si.on_wait else []
                cap = 2 if getattr(inst, "opcode", None) == "EventSemaphore" else 1
                if len(waits) <= cap:
                    if new_insts is not None:
                        new_insts.append(inst)
                    continue
                if new_insts is None:
                    new_insts = list(bb.instructions[:idx])
                keep, excess = waits[-cap:], waits[:-cap]
                for w in excess:
                    n += 1
                    nop = mybir.InstNoOp(name=f"I-wsplit-{n}-{inst.name}", ins=[], outs=[])
                    nop.engine = inst.engine
                    nop.sync_info = mybir.SyncInfo(on_wait=[w], on_update=[])
                    new_insts.append(nop)
                si.on_wait = keep
                new_insts.append(inst)
            if new_insts is not None:
                bb.instructions = new_insts
    return n


def _build_nc():
    from contextlib import ExitStack

    import concourse.bass as bass
    import concourse.tile as tile
    from concourse import mybir

    F32 = mybir.dt.float32
    BF16 = mybir.dt.bfloat16
    I16 = mybir.dt.int16
    AF = mybir.ActivationFunctionType
    ALU = mybir.AluOpType
    AX = mybir.AxisListType

    nc = bass.Bass("TRN2", target_bir_lowering=False, debug=False)
    z1 = nc.declare_dram_parameter("z1", [N, D], F32, isOutput=False).ap()
    z2 = nc.declare_dram_parameter("z2", [N, D], F32, isOutput=False).ap()
    z1q = nc.declare_dram_parameter("z1q", [NQ, D], F32, isOutput=False).ap()
    z2q = nc.declare_dram_parameter("z2q", [NQ, D], F32, isOutput=False).ap()
    out = nc.declare_dram_parameter("out", [P, QT], F32, isOutput=True).ap()

    with tile.TileContext(nc) as tc, ExitStack() as ctx:
        persist = ctx.enter_context(tc.tile_pool(name="persist", bufs=1))
        stage_p = ctx.enter_context(tc.tile_pool(name="stage", bufs=2))
        sq_p = ctx.enter_context(tc.tile_pool(name="sq", bufs=1))
        norm_p = ctx.enter_context(tc.tile_pool(name="norms", bufs=2))
        nbg_p = ctx.enter_context(tc.tile_pool(name="nbg", bufs=2))
        i16_p = ctx.enter_context(tc.tile_pool(name="i16", bufs=2))
        work_p = ctx.enter_context(tc.tile_pool(name="work", bufs=4))
        ps_p = ctx.enter_context(tc.tile_pool(name="ps", bufs=2, space="PSUM"))

        z1T = persist.tile([P, N], BF16, tag="z1T")
        z2T = persist.tile([P, N], BF16, tag="z2T")
        z1qT = persist.tile([P, NQ], BF16, tag="z1qT")
        z2qn = persist.tile([P, NQ], F32, tag="z2qn")
        pos_raw = persist.tile([P, QT], F32, tag="pos")
        d_raw = persist.tile([P, QT], F32, tag="draw")
        part_a = persist.tile([P, QT * NCHUNKS], F32, tag="parta")
        part_d = persist.tile([P, QT * NCHUNKS], F32, tag="partd")

        # ---------------- PE warm-up ----------------
        # ~28 back-to-back dummy matmuls (~6-12us) release the PE HAM clock
        # gate (cold=1.2GHz -> warm=2.4GHz) while the prologue DMAs run.
        warmA = persist.tile([P, P], BF16, tag="warmA")
        warmB = persist.tile([P, SUB], BF16, tag="warmB")
        nc.gpsimd.memset(warmA[:, :], 0.0)
        nc.gpsimd.memset(warmB[:, :], 0.0)
        ps_w = ps_p.tile([P, CHUNK], F32, tag="ps")
        for i in range(NWARM):
            nc.tensor.matmul(
                ps_w[:, (i % 4) * SUB:((i % 4) + 1) * SUB],
                lhsT=warmA, rhs=warmB, start=True, stop=True,
            )

        def rsqrt_newton(ssq, ntiles):
            """r = 1/sqrt(ssq) elementwise over [P, ntiles]; ACT sqrt seed
            + DVE Newton step (stays in the natural_log_exp ACT table set)."""
            r0 = norm_p.tile([P, GROUP], F32, tag="r0")
            t1 = norm_p.tile([P, GROUP], F32, tag="t1")
            nc.scalar.activation(r0[:, :ntiles], ssq[:, :ntiles], AF.Ln)
            nc.scalar.activation(r0[:, :ntiles], r0[:, :ntiles], AF.Exp,
                                 bias=0.0, scale=-0.5)
            nc.vector.tensor_mul(t1[:, :ntiles], r0[:, :ntiles], r0[:, :ntiles])
            nc.vector.tensor_mul(t1[:, :ntiles], t1[:, :ntiles], ssq[:, :ntiles])
            nc.vector.tensor_scalar(
                out=t1[:, :ntiles], in0=t1[:, :ntiles],
                scalar1=-0.5, scalar2=1.5, op0=ALU.mult, op1=ALU.add,
            )
            nc.vector.tensor_mul(r0[:, :ntiles], r0[:, :ntiles], t1[:, :ntiles])
            return r0

        def load_group(src, row0, ntiles):
            """DMA ntiles row tiles to staging; compute 1/norm per row.
            Squares on GpSimd (batched), one batched 3D reduce on DVE."""
            stage = stage_p.tile([P, GROUP, P], F32, tag="stage")
            nc.sync.dma_start(
                out=stage[:, :ntiles, :],
                in_=src[row0:row0 + ntiles * P, :].rearrange("(t p) d -> p t d", p=P),
            )
            sqg = sq_p.tile([P, GROUP, P], F32, tag="sqg")
            nc.gpsimd.tensor_mul(sqg[:, :ntiles, :], stage[:, :ntiles, :],
                                 stage[:, :ntiles, :])
            ssq = norm_p.tile([P, GROUP], F32, tag="ssq")
            nc.vector.tensor_reduce(
                out=ssq[:, :ntiles], in_=sqg[:, :ntiles, :], axis=AX.X, op=ALU.add)
            return stage, rsqrt_newton(ssq, ntiles)

        def normalize_group(stage, r, ntiles, dst=None, dtype=BF16, tag="nbg"):
            """GpSimd per-tile scale+cast into one contiguous buffer."""
            if dst is None:
                dst = nbg_p.tile([P, GROUP * P], dtype, tag=tag)
            for t in range(ntiles):
                nc.gpsimd.tensor_scalar_mul(
                    dst[:, t * P:(t + 1) * P], stage[:, t, :], r[:, t:t + 1])
            return dst

        def transpose_group(nbg, dst_T, col0, ntiles):
            """one batched DMA-xbar transpose: [P, ntiles*P] -> ntiles tiles."""
            dst3 = dst_T[:, col0:col0 + ntiles * P].rearrange(
                "p (t d) -> p t d", d=P)
            nc.sync.dma_start_transpose(dst3, nbg[:, :ntiles * P])

        def exp_unit(q, ck, keysT, koff):
            """4 matmuls filling a 4-bank PSUM slot; exp+row-sum split
            between ACT (cols 0:ACT_W) and DVE bit-trick (cols ACT_W:2048)."""
            ps = ps_p.tile([P, CHUNK], F32, tag="ps")
            kxm = z1qT[:, q * P:(q + 1) * P]
            for j in range(4):
                nc.tensor.matmul(
                    ps[:, j * SUB:(j + 1) * SUB],
                    lhsT=kxm,
                    rhs=keysT[:, koff + j * SUB: koff + (j + 1) * SUB],
                    start=True, stop=True,
                )
            slot = q * NCHUNKS + ck
            # ACT: fused exp + row-sum, in-place over PSUM
            nc.scalar.activation(
                ps[:, :ACT_W], ps[:, :ACT_W], AF.Exp, bias=0.0, scale=1.0 / TAU,
                accum_out=part_a[:, slot:slot + 1],
            )
            # DVE: op1 builds the bf16 bit patterns of ~exp(s/tau)
            i16 = i16_p.tile([P, DVE_W], I16, tag="i16")
            nc.vector.tensor_scalar(
                out=i16[:, :], in0=ps[:, ACT_W:CHUNK],
                scalar1=A_EXP, scalar2=B_EXP, op0=ALU.mult, op1=ALU.add,
            )
            # DVE: op2 re-reads them as bf16 (4x mode) and row-sums via accum
            bfv = i16[:, :].bitcast(BF16)
            nc.vector.tensor_scalar(
                out=bfv, in0=bfv, scalar1=1.0, scalar2=None,
                op0=ALU.mult, op1=ALU.add,
                accum_out=part_d[:, slot:slot + 1],
            )

        # ---------------- prologue: what the exp stream needs ----------
        z1qn = persist.tile([P, NQ], BF16, tag="z1qn")
        z1qnf = persist.tile([P, NQ], F32, tag="z1qnf")

        stage, r = load_group(z1q, 0, QT)
        stage3 = stage  # [P, GROUP, P]; only first QT tiles valid
        normalize_group(stage, r, QT, dst=z1qn, tag="z1qn")
        transpose_group(z1qn, z1qT, 0, QT)
        # f32 normalized z1q rows, for pos (deferred consumer)
        normalize_group(stage, r, QT, dst=z1qnf, dtype=F32, tag="z1qnf")

        def deferred_qprep():
            """z2q chain + pos + d: runs in engine slack under the exps."""
            stg, rq = load_group(z2q, 0, QT)
            normalize_group(stg, rq, QT, dst=z2qn, dtype=F32, tag="z2qn")
            # d_raw[:, q] = sum_d bf16(z1n)^2 (matches the PE diag dot)
            sqq = sq_p.tile([P, GROUP, P], F32, tag="qsq")
            z1qn3 = z1qn[:, :].rearrange("p (t d) -> p t d", d=P)
            nc.gpsimd.tensor_mul(sqq[:, :QT, :], z1qn3, z1qn3)
            nc.vector.tensor_reduce(
                out=d_raw[:, :], in_=sqq[:, :QT, :], axis=AX.X, op=ALU.add)
            # pos_raw[:, q] = sum_d z1n * z2n (f32)
            mbq = sq_p.tile([P, GROUP, P], F32, tag="qmb")
            z1qnf3 = z1qnf[:, :].rearrange("p (t d) -> p t d", d=P)
            z2qn3 = z2qn[:, :].rearrange("p (t d) -> p t d", d=P)
            nc.gpsimd.tensor_mul(mbq[:, :QT, :], z1qnf3, z2qn3)
            nc.vector.tensor_reduce(
                out=pos_raw[:, :], in_=mbq[:, :QT, :], axis=AX.X, op=ALU.add)

        # ---------------- steady state ----------------
        # groups of 32 row tiles; z2 -> chunks 0..7, z1 -> chunks 8..15.
        groups = []
        for m, (src, dst_T) in enumerate(((z2, z2T), (z1, z1T))):
            for g in range(NGRP):
                groups.append((src, dst_T, g, m * (NGRP * 2) + g * 2))

        # prime group 0
        src0, dstT0, g0, _ = groups[0]
        stage_cur, r_cur = load_group(src0, g0 * GROUP * P, GROUP)
        nbg = normalize_group(stage_cur, r_cur, GROUP)
        transpose_group(nbg, dstT0, g0 * GROUP * P, GROUP)

        for gi, (src, dst_T, g, ckbase) in enumerate(groups):
            # prep the NEXT group (overlaps this group's exp stream)
            if gi + 1 < len(groups):
                nsrc, ndst, ng, _ = groups[gi + 1]
                stage_nxt, r_nxt = load_group(nsrc, ng * GROUP * P, GROUP)
                nbg_nxt = normalize_group(stage_nxt, r_nxt, GROUP)
                transpose_group(nbg_nxt, ndst, ng * GROUP * P, GROUP)
            if gi == 0:
                # fill engine slack under group 0's exps
                deferred_qprep()

            # 32 exp units for this group's two 2048-key chunks
            for half in range(2):
                ck = ckbase + half
                koff = (g * GROUP + half * (GROUP // 2)) * P
                for q in range(QT):
                    exp_unit(q, ck, dst_T, koff)

        # ---------------- epilogue: per-row losses ----------------
        S_a = work_p.tile([P, QT], F32, tag="sa")
        S_d = work_p.tile([P, QT], F32, tag="sd")
        nc.vector.tensor_reduce(
            out=S_a[:, :],
            in_=part_a[:, :].rearrange("p (q c) -> p q c", c=NCHUNKS),
            axis=AX.X, op=ALU.add,
        )
        nc.vector.tensor_reduce(
            out=S_d[:, :],
            in_=part_d[:, :].rearrange("p (q c) -> p q c", c=NCHUNKS),
            axis=AX.X, op=ALU.add,
        )
        S_raw = work_p.tile([P, QT], F32, tag="sraw")
        nc.vector.tensor_add(S_raw[:, :], S_a[:, :], S_d[:, :])

        # diagonal corrections: ACT form for q<QSPLIT, DVE bit-trick form after
        exp_da = work_p.tile([P, QT], F32, tag="expda")
        nc.scalar.activation(exp_da[:, :], d_raw[:, :], AF.Exp,
                             bias=0.0, scale=1.0 / TAU)
        i16d = work_p.tile([P, QT], I16, tag="i16d")
        nc.vector.tensor_scalar(
            out=i16d[:, :], in0=d_raw[:, :],
            scalar1=A_EXP, scalar2=B_EXP, op0=ALU.mult, op1=ALU.add,
        )
        exp_dd = work_p.tile([P, QT], F32, tag="expdd")
        nc.vector.tensor_copy(exp_dd[:, :], i16d[:, :].bitcast(BF16))

        s_corr = work_p.tile([P, QT], F32, tag="scorr")
        nc.vector.tensor_sub(s_corr[:, :QSPLIT], S_raw[:, :QSPLIT],
                             exp_da[:, :QSPLIT])
        nc.vector.tensor_sub(s_corr[:, QSPLIT:], S_raw[:, QSPLIT:],
                             exp_dd[:, QSPLIT:])

        lse = work_p.tile([P, QT], F32, tag="lse")
        nc.scalar.activation(lse[:, :], s_corr[:, :], AF.Ln)
        negpos = work_p.tile([P, QT], F32, tag="negpos")
        nc.vector.tensor_scalar(
            out=negpos[:, :], in0=pos_raw[:, :],
            scalar1=-1.0 / TAU, scalar2=None, op0=ALU.mult,
        )
        loss = work_p.tile([P, QT], F32, tag="loss")
        nc.vector.tensor_add(loss[:, :], lse[:, :], negpos[:, :])
        nc.sync.dma_start(out=out[:, :], in_=loss[:, :])

    _split_excess_waits(nc, mybir)
    return nc


def _get_nc():
    if "nc" not in _CACHE:
        _CACHE["nc"] = _build_nc()
    return _CACHE["nc"]


def kernel(z1, z2):
    from concourse.bass_utils import run_bass_kernel_spmd

    z1 = np.ascontiguousarray(np.asarray(z1, dtype=np.float32))
    z2 = np.ascontiguousarray(np.asarray(z2, dtype=np.float32))
    assert z1.shape == (N, D) and z2.shape == (N, D)

    nc = _get_nc()
    in_maps = [
        {
            "z1": z1,
            "z2": z2,
            "z1q": np.ascontiguousarray(z1[c * NQ:(c + 1) * NQ]),
            "z2q": np.ascontiguousarray(z2[c * NQ:(c + 1) * NQ]),
        }
        for c in range(NCORES)
    ]
    trace = bool(int(os.environ.get("TRNLOSS_TRACE", "0")))
    res = run_bass_kernel_spmd(nc, in_maps, core_ids=list(range(NCORES)), trace=trace)
    if trace:
        _CACHE["exec_time_ns"] = res.exec_time_ns
        print(f"HW exec time: {res.exec_time_ns} ns")
    total = 0.0
    for c in range(NCORES):
        total += res.results[c]["out"].astype(np.float64).sum()
    return np.float32(total / N)


# revision 12
# speedup vs baseline: 1.5931x; 1.5931x over previous
"""NodeContrastiveLoss on 8 Trainium2 NeuronCores (Bass/Tile), v2.

loss = mean_i[ -(z1n_i . z2n_i)/tau
               + log( sum_j exp((z1n_i . z2n_j)/tau)
                    + sum_{j!=i} exp((z1n_i . z1n_j)/tau) ) ]

Sharding: z1 query rows split 8 ways (2048 rows/core); every core builds the
full normalized key matrices z1n^T, z2n^T in SBUF (bf16) and computes its row
block of the similarity logits with PE matmuls.

v2 change vs v1: the exp+row-sum of each [128 x 2048] PSUM chunk is SPLIT
between two engines working concurrently on disjoint column ranges:
  - ACT: native Exp activation with accum_out on cols [0:ACT_W]
  - DVE: Schraudolph-style bf16 bit-trick exp on cols [ACT_W:2048]:
      op1: int16 <- A_EXP*s + B_EXP   (1x, PSUM f32 -> SBUF int16)
      op2: bitcast int16 as bf16, re-copy with accum_out row-sum (4x mode)
    bitcast(int16(A*s+B)) == 2^(s*log2e/tau + frac-interp) ~ exp(s/tau) with
    |rel err| <= 4.2% per element and ~0 mean bias (B calibrated); errors
    wash out in the 32768-term row sums and the 16384-row mean.
Key prep (squares for row norms, normalize-scale to bf16) moves to GpSimd +
batched DVE reduces so the DVE's cycles go to the exp stream. A burst of
dummy matmuls at t=0 warms the PE HAM clock gate (cold PE = 1.2 GHz would
otherwise gate the pipeline).

The z1-z1 diagonal is removed by subtracting exp(||z1n_i||^2/tau) computed
from the same bf16 values the PE consumes - via ACT Exp for q-tiles whose
diagonal falls in the ACT columns (q<10) and via the identical DVE bit-trick
transform for q>=10 (ACT_W=1280 is a multiple of 128 so each q-tile's
diagonal lies entirely on one side).
"""

import os
import numpy as np

N, D = 16384, 128
TAU = 0.07
NCORES = 8
NQ = N // NCORES          # 2048 query rows per core
P = 128
QT = NQ // P              # 16 query tiles per core
GROUP = 32                # row tiles per staging group (4096 rows)
CHUNK = 2048              # keys per chunk (4 PSUM banks)
SUB = 512                 # matmul moving free dim (1 PSUM bank)
NGRP = N // (GROUP * P)   # 4 groups per key matrix
NCHUNKS = 2 * N // CHUNK  # 16 global chunks per q-tile (z2 then z1)

ACT_W = 1408              # ACT exp columns per chunk (multiple of 128)
DVE_W = CHUNK - ACT_W     # 640 DVE bit-trick columns per chunk
QSPLIT = ACT_W // P       # q-tiles 0..QSPLIT-1 have their diagonal in ACT cols

LOG2E = 1.4426950408889634
A_EXP = 128.0 * LOG2E / TAU          # 2638.0709...
B_EXP = 16248.888                    # 128*127 - 7.112 (zero-bias calibrated)

NWARM = 28                # dummy matmuls to warm the PE HAM clock gate

_CACHE = {}


def _split_excess_waits(nc, mybir):
    """walrus in this env supports 1 sync-wait per instruction (2 for
    EventSemaphore); move excess waits onto injected same-engine NoOps."""
    n = 0
    for f in nc.m.functions:
        for bb in f.blocks:
            new_insts = None
            for idx, inst in enumerate(bb.instructions):
                si = getattr(inst, "sync_info", None)
                waits = list(si.on_wait) if si is not None and si.on_wait else []
                cap = 2 if getattr(inst, "opcode", None) == "EventSemaphore" else 1
                if len(waits) <= cap:
                    if new_insts is not None:
                        new_insts.append(inst)
                    continue
                if new_insts is None:
                    new_insts = list(bb.instructions[:idx])
                keep, excess = waits[-cap:], waits[:-cap]
                for w in excess:
                    n += 1
                    nop = mybir.InstNoOp(name=f"I-wsplit-{n}-{inst.name}", ins=[], outs=[])
                    nop.engine = inst.engine
                    nop.sync_info = mybir.SyncInfo(on_wait=[w], on_update=[])
                    new_insts.append(nop)
                si.on_wait = keep
                new_insts.append(inst)
            if new_insts is not None:
                bb.instructions = new_insts
    return n


def _build_nc():
    from contextlib import ExitStack

    import concourse.bass as bass
    import concourse.tile as tile
    from concourse import mybir

    F32 = mybir.dt.float32
    BF16 = mybir.dt.bfloat16
    I16 = mybir.dt.int16
    AF = mybir.ActivationFunctionType
    ALU = mybir.AluOpType
    AX = mybir.AxisListType

    nc = bass.Bass("TRN2", target_bir_lowering=False, debug=False)
    z1 = nc.declare_dram_parameter("z1", [N, D], F32, isOutput=False).ap()
    z2 = nc.declare_dram_parameter("z2", [N, D], F32, isOutput=False).ap()
    z1q = nc.declare_dram_parameter("z1q", [NQ, D], F32, isOutput=False).ap()
    z2q = nc.declare_dram_parameter("z2q", [NQ, D], F32, isOutput=False).ap()
    out = nc.declare_dram_parameter("out", [P, QT], F32, isOutput=True).ap()

    with tile.TileContext(nc) as tc, ExitStack() as ctx:
        persist = ctx.enter_context(tc.tile_pool(name="persist", bufs=1))
        stage_p = ctx.enter_context(tc.tile_pool(name="stage", bufs=2))
        sq_p = ctx.enter_context(tc.tile_pool(name="sq", bufs=1))
        qsq_p = ctx.enter_context(tc.tile_pool(name="qsq", bufs=1))
        norm_p = ctx.enter_context(tc.tile_pool(name="norms", bufs=2))
        nbg_p = ctx.enter_context(tc.tile_pool(name="nbg", bufs=2))
        i16_p = ctx.enter_context(tc.tile_pool(name="i16", bufs=2))
        work_p = ctx.enter_context(tc.tile_pool(name="work", bufs=4))
        ps_p = ctx.enter_context(tc.tile_pool(name="ps", bufs=2, space="PSUM"))

        z1T = persist.tile([P, N], BF16, tag="z1T")
        z2T = persist.tile([P, N], BF16, tag="z2T")
        z1qT = persist.tile([P, NQ], BF16, tag="z1qT")
        z2qn = persist.tile([P, NQ], F32, tag="z2qn")
        pos_raw = persist.tile([P, QT], F32, tag="pos")
        d_raw = persist.tile([P, QT], F32, tag="draw")
        part_a = persist.tile([P, QT * NCHUNKS], F32, tag="parta")
        # per-q-tile bf16 elementwise accumulators for the DVE exp stream:
        # acc_d[q][p, c] = sum over the 16 chunks of that chunk's bf16
        # exp value at column c. Accumulated via tensor_tensor add (2x_1P
        # DVE mode); reduced to row sums once in the epilogue.
        acc_d = persist.tile([P, QT * DVE_W], BF16, tag="accd")

        # ---------------- PE warm-up ----------------
        # ~28 back-to-back dummy matmuls (~6-12us) release the PE HAM clock
        # gate (cold=1.2GHz -> warm=2.4GHz) while the prologue DMAs run.
        warmA = persist.tile([P, P], BF16, tag="warmA")
        warmB = persist.tile([P, SUB], BF16, tag="warmB")
        nc.gpsimd.memset(warmA[:, :], 0.0)
        nc.gpsimd.memset(warmB[:, :], 0.0)
        nc.gpsimd.memset(acc_d[:, :], 0.0)
        ps_w = ps_p.tile([P, CHUNK], F32, tag="ps")
        for i in range(NWARM):
            nc.tensor.matmul(
                ps_w[:, (i % 4) * SUB:((i % 4) + 1) * SUB],
                lhsT=warmA, rhs=warmB, start=True, stop=True,
            )

        def rsqrt_newton(ssq, ntiles):
            """r = 1/sqrt(ssq) elementwise over [P, ntiles]; ACT sqrt seed
            + DVE Newton step (stays in the natural_log_exp ACT table set)."""
            r0 = norm_p.tile([P, GROUP], F32, tag="r0")
            t1 = norm_p.tile([P, GROUP], F32, tag="t1")
            nc.scalar.activation(r0[:, :ntiles], ssq[:, :ntiles], AF.Ln)
            nc.scalar.activation(r0[:, :ntiles], r0[:, :ntiles], AF.Exp,
                                 bias=0.0, scale=-0.5)
            nc.vector.tensor_mul(t1[:, :ntiles], r0[:, :ntiles], r0[:, :ntiles])
            nc.vector.tensor_mul(t1[:, :ntiles], t1[:, :ntiles], ssq[:, :ntiles])
            nc.vector.tensor_scalar(
                out=t1[:, :ntiles], in0=t1[:, :ntiles],
                scalar1=-0.5, scalar2=1.5, op0=ALU.mult, op1=ALU.add,
            )
            nc.vector.tensor_mul(r0[:, :ntiles], r0[:, :ntiles], t1[:, :ntiles])
            return r0

        def load_group(src, row0, ntiles):
            """DMA ntiles row tiles to staging; compute 1/norm per row.
            Squares on GpSimd (batched), one batched 3D reduce on DVE."""
            stage = stage_p.tile([P, GROUP, P], F32, tag="stage")
            nc.sync.dma_start(
                out=stage[:, :ntiles, :],
                in_=src[row0:row0 + ntiles * P, :].rearrange("(t p) d -> p t d", p=P),
            )
            sqg = sq_p.tile([P, GROUP, P], F32, tag="sqg")
            nc.gpsimd.tensor_mul(sqg[:, :ntiles, :], stage[:, :ntiles, :],
                                 stage[:, :ntiles, :])
            ssq = norm_p.tile([P, GROUP], F32, tag="ssq")
            nc.vector.tensor_reduce(
                out=ssq[:, :ntiles], in_=sqg[:, :ntiles, :], axis=AX.X, op=ALU.add)
            return stage, rsqrt_newton(ssq, ntiles)

        def normalize_group(stage, r, ntiles, dst=None, dtype=BF16, tag="nbg"):
            """DVE per-tile scale+cast (2x_2P mode) into one contiguous
            buffer. (GpSimd measured 2us/tile for this - keep it on DVE.)"""
            if dst is None:
                dst = nbg_p.tile([P, GROUP * P], dtype, tag=tag)
            for t in range(ntiles):
                nc.vector.tensor_scalar_mul(
                    dst[:, t * P:(t + 1) * P], stage[:, t, :], r[:, t:t + 1])
            return dst

        def transpose_group(nbg, dst_T, col0, ntiles):
            """one batched DMA-xbar transpose: [P, ntiles*P] -> ntiles tiles."""
            dst3 = dst_T[:, col0:col0 + ntiles * P].rearrange(
                "p (t d) -> p t d", d=P)
            nc.sync.dma_start_transpose(dst3, nbg[:, :ntiles * P])

        def exp_unit(q, ck, keysT, koff):
            """4 matmuls filling a 4-bank PSUM slot; exp+row-sum split
            between ACT (cols 0:ACT_W) and DVE bit-trick (cols ACT_W:2048)."""
            ps = ps_p.tile([P, CHUNK], F32, tag="ps")
            kxm = z1qT[:, q * P:(q + 1) * P]
            for j in range(4):
                nc.tensor.matmul(
                    ps[:, j * SUB:(j + 1) * SUB],
                    lhsT=kxm,
                    rhs=keysT[:, koff + j * SUB: koff + (j + 1) * SUB],
                    start=True, stop=True,
                )
            slot = q * NCHUNKS + ck
            # ACT: fused exp + row-sum, in-place over PSUM
            nc.scalar.activation(
                ps[:, :ACT_W], ps[:, :ACT_W], AF.Exp, bias=0.0, scale=1.0 / TAU,
                accum_out=part_a[:, slot:slot + 1],
            )
            # DVE: op1 builds the bf16 bit patterns of ~exp(s/tau)
            i16 = i16_p.tile([P, DVE_W], I16, tag="i16")
            nc.vector.tensor_scalar(
                out=i16[:, :], in0=ps[:, ACT_W:CHUNK],
                scalar1=A_EXP, scalar2=B_EXP, op0=ALU.mult, op1=ALU.add,
            )
            # DVE: op2 accumulates them (as bf16) into this q-tile's
            # elementwise accumulator - tensor_tensor add runs in 2x_1P mode
            acc = acc_d[:, q * DVE_W:(q + 1) * DVE_W]
            nc.vector.tensor_add(acc, acc, i16[:, :].bitcast(BF16))

        # ---------------- prologue: what the exp stream needs ----------
        z1qn = persist.tile([P, NQ], BF16, tag="z1qn")
        z1qnf = persist.tile([P, NQ], F32, tag="z1qnf")

        stage, r = load_group(z1q, 0, QT)
        stage3 = stage  # [P, GROUP, P]; only first QT tiles valid
        normalize_group(stage, r, QT, dst=z1qn, tag="z1qn")
        transpose_group(z1qn, z1qT, 0, QT)
        # f32 normalized z1q rows, for pos (deferred consumer)
        normalize_group(stage, r, QT, dst=z1qnf, dtype=F32, tag="z1qnf")

        def deferred_qprep():
            """z2q chain + pos + d: runs in engine slack under the exps."""
            stg, rq = load_group(z2q, 0, QT)
            normalize_group(stg, rq, QT, dst=z2qn, dtype=F32, tag="z2qn")
            # d_raw[:, q] = sum_d bf16(z1n)^2 (matches the PE diag dot)
            sqq = qsq_p.tile([P, QT, P], F32, tag="qsq")
            z1qn3 = z1qn[:, :].rearrange("p (t d) -> p t d", d=P)
            nc.gpsimd.tensor_mul(sqq[:, :, :], z1qn3, z1qn3)
            nc.vector.tensor_reduce(
                out=d_raw[:, :], in_=sqq[:, :, :], axis=AX.X, op=ALU.add)
            # pos_raw[:, q] = sum_d z1n * z2n (f32)
            mbq = qsq_p.tile([P, QT, P], F32, tag="qsq")
            z1qnf3 = z1qnf[:, :].rearrange("p (t d) -> p t d", d=P)
            z2qn3 = z2qn[:, :].rearrange("p (t d) -> p t d", d=P)
            nc.gpsimd.tensor_mul(mbq[:, :, :], z1qnf3, z2qn3)
            nc.vector.tensor_reduce(
                out=pos_raw[:, :], in_=mbq[:, :, :], axis=AX.X, op=ALU.add)

        # ---------------- steady state ----------------
        # groups of 32 row tiles; z2 -> chunks 0..7, z1 -> chunks 8..15.
        groups = []
        for m, (src, dst_T) in enumerate(((z2, z2T), (z1, z1T))):
            for g in range(NGRP):
                groups.append((src, dst_T, g, m * (NGRP * 2) + g * 2))

        # prime group 0
        src0, dstT0, g0, _ = groups[0]
        stage_cur, r_cur = load_group(src0, g0 * GROUP * P, GROUP)
        nbg = normalize_group(stage_cur, r_cur, GROUP)
        transpose_group(nbg, dstT0, g0 * GROUP * P, GROUP)

        for gi, (src, dst_T, g, ckbase) in enumerate(groups):
            # prep the NEXT group (overlaps this group's exp stream)
            if gi + 1 < len(groups):
                nsrc, ndst, ng, _ = groups[gi + 1]
                stage_nxt, r_nxt = load_group(nsrc, ng * GROUP * P, GROUP)
                nbg_nxt = normalize_group(stage_nxt, r_nxt, GROUP)
                transpose_group(nbg_nxt, ndst, ng * GROUP * P, GROUP)
            if gi == 0:
                # fill engine slack under group 0's exps
                deferred_qprep()

            # 32 exp units for this group's two 2048-key chunks; q outer so
            # consecutive chunk pairs share the same stationary lhsT
            for q in range(QT):
                for half in range(2):
                    ck = ckbase + half
                    koff = (g * GROUP + half * (GROUP // 2)) * P
                    exp_unit(q, ck, dst_T, koff)

        # ---------------- epilogue: per-row losses ----------------
        S_a = work_p.tile([P, QT], F32, tag="sa")
        S_d = work_p.tile([P, QT], F32, tag="sd")
        nc.vector.tensor_reduce(
            out=S_a[:, :],
            in_=part_a[:, :].rearrange("p (q c) -> p q c", c=NCHUNKS),
            axis=AX.X, op=ALU.add,
        )
        nc.vector.tensor_reduce(
            out=S_d[:, :],
            in_=acc_d[:, :].rearrange("p (q c) -> p q c", c=DVE_W),
            axis=AX.X, op=ALU.add,
        )
        S_raw = work_p.tile([P, QT], F32, tag="sraw")
        nc.vector.tensor_add(S_raw[:, :], S_a[:, :], S_d[:, :])

        # diagonal corrections: ACT form for q<QSPLIT, DVE bit-trick form after
        exp_da = work_p.tile([P, QT], F32, tag="expda")
        nc.scalar.activation(exp_da[:, :], d_raw[:, :], AF.Exp,
                             bias=0.0, scale=1.0 / TAU)
        i16d = work_p.tile([P, QT], I16, tag="i16d")
        nc.vector.tensor_scalar(
            out=i16d[:, :], in0=d_raw[:, :],
            scalar1=A_EXP, scalar2=B_EXP, op0=ALU.mult, op1=ALU.add,
        )
        exp_dd = work_p.tile([P, QT], F32, tag="expdd")
        nc.vector.tensor_copy(exp_dd[:, :], i16d[:, :].bitcast(BF16))

        s_corr = work_p.tile([P, QT], F32, tag="scorr")
        nc.vector.tensor_sub(s_corr[:, :QSPLIT], S_raw[:, :QSPLIT],
                             exp_da[:, :QSPLIT])
        nc.vector.tensor_sub(s_corr[:, QSPLIT:], S_raw[:, QSPLIT:],
                             exp_dd[:, QSPLIT:])

        lse = work_p.tile([P, QT], F32, tag="lse")
        nc.scalar.activation(lse[:, :], s_corr[:, :], AF.Ln)
        negpos = work_p.tile([P, QT], F32, tag="negpos")
        nc.vector.tensor_scalar(
            out=negpos[:, :], in0=pos_raw[:, :],
            scalar1=-1.0 / TAU, scalar2=None, op0=ALU.mult,
        )
        loss = work_p.tile([P, QT], F32, tag="loss")
        nc.vector.tensor_add(loss[:, :], lse[:, :], negpos[:, :])
        nc.sync.dma_start(out=out[:, :], in_=loss[:, :])

    _split_excess_waits(nc, mybir)
    return nc


def _get_nc():
    if "nc" not in _CACHE:
        _CACHE["nc"] = _build_nc()
    return _CACHE["nc"]


def kernel(z1, z2):
    from concourse.bass_utils import run_bass_kernel_spmd

    z1 = np.ascontiguousarray(np.asarray(z1, dtype=np.float32))
    z2 = np.ascontiguousarray(np.asarray(z2, dtype=np.float32))
    assert z1.shape == (N, D) and z2.shape == (N, D)

    nc = _get_nc()
    in_maps = [
        {
            "z1": z1,
            "z2": z2,
            "z1q": np.ascontiguousarray(z1[c * NQ:(c + 1) * NQ]),
            "z2q": np.ascontiguousarray(z2[c * NQ:(c + 1) * NQ]),
        }
        for c in range(NCORES)
    ]
    trace = bool(int(os.environ.get("TRNLOSS_TRACE", "0")))
    res = run_bass_kernel_spmd(nc, in_maps, core_ids=list(range(NCORES)), trace=trace)
    if trace:
        _CACHE["exec_time_ns"] = res.exec_time_ns
        print(f"HW exec time: {res.exec_time_ns} ns")
    total = 0.0
    for c in range(NCORES):
        total += res.results[c]["out"].astype(np.float64).sum()
    return np.float32(total / N)
